# revision 36
# baseline (speedup 1.0000x reference)
"""Complex coherency loss, distributed over 8 TRN2 NeuronCores.

Data-parallel over batch: core b handles batch element b; the host sums
the per-chunk partial sums and finishes the mean.

v4 design (bf16 plane-separate; replaces the fp8 DoubleRow v3):
  - measured v3 showed fp8 elementwise is slow on HW (no DVE 2x mode for
    1-byte dtypes, GpSimd mul ~6us/2048cols) and starved fp8 matmuls
    never reach the DoubleRow rate.  bf16 keeps DVE in 2x mode.
  - plane-separate layout: partition p = c + 64h (h = l-half), plane
    tiles PR/PI/TR/TI [128, 1024] per group.  All four products
    (pr*tr, pi*ti, pi*tr, pr*ti) are direct tile muls - no swapped copy
    of T and no extra HBM traffic.
  - 8 moving streams per group into the PE; selector weights route
    stream q to PSUM row 2q+h (m4 with weight -1), accumulating m12,
    m34, A, B in natural l order within each half.
  - PSUM [8, 1024] f32 -> DVE/ACT/GPS-split bf16 relay -> 2 scatter
    DMAs per group into a flat DRAM scratch [4, L+4].
  - stage 2 in 4 chunks of 4096 windows ([64, 4, 68] gathers): flat
    5-tap window adds, ratio, sqrt-accum.  Two chunks fire mid-loop.
  - PE p-state warmup matmuls during the load window.
"""

import numpy as np
import ml_dtypes

import concourse.bass as bass
import concourse.bacc as bacc
import concourse.mybir as mybir
import concourse.tile as tile
from concourse.bass_utils import run_bass_kernel_spmd

B, C, L = 8, 64, 16384
K = 5
P = 128
NVALID = L - K + 1        # 16380 valid windows per batch element

NG = 8                    # groups
FD = 1024                 # plane cols per group (per half)
HL = L // 2               # 8192 l's per half
SCR_W = L + 4             # scratch cols per quantity (4 zero-pad cols)
NQ = 4                    # m12, m34, A, B
NS = 8                    # moving streams per group
CH = 512                  # matmul chunk (one psum bank of f32)
NPP = 64                  # l's per partition in stage 2
CK = 64 * NPP             # 4096 l's per stage-2 chunk
HALO = 4

F32 = mybir.dt.float32
BF16 = mybir.dt.bfloat16
NP_BF16 = ml_dtypes.bfloat16

PROFILE = False
TRACE_DIR = None
LAST_RESULT = None

# stream -> (quantity row q, sign): m1,m2->m12; m3,+ m4,- ->m34; squares
STREAM_Q = [(0, 1.0), (0, 1.0), (1, 1.0), (1, -1.0),
            (2, 1.0), (2, 1.0), (3, 1.0), (3, 1.0)]


def _selector_weights() -> np.ndarray:
    """Per-stream selectors [128, NS * 8] bf16: stream s routes
    partition p = c + 64h into PSUM row 4h + q_s with weight sign_s
    (h-major rows so each half drains as a contiguous row block)."""
    w = np.zeros((P, NS, 8), dtype=np.float32)
    p = np.arange(P)
    h = p // 64
    for s, (q, sign) in enumerate(STREAM_Q):
        w[p, s, 4 * h + q] = sign
    return w.reshape(P, NS * 8).astype(NP_BF16)


def _fuse_ldweights(nc) -> None:
    """Re-fuse Ldweights+Matmult into self-loading matmuls (tile
    legalization splits them; merging back halves PE instruction count
    and sidesteps walrus restrictions on standalone Ldweights)."""
    rename = {}
    for fn in nc.m.functions:
        for blk in fn.blocks:
            ins = list(blk.instructions)
            out, pending = [], None
            for inst in ins:
                if inst.opcode == "Ldweights":
                    assert pending is None
                    pending = inst
                    continue
                if inst.opcode == "Matmult" and pending is not None:
                    lsi, si = pending.sync_info, inst.sync_info
                    lw_waits = list(lsi.on_wait) if lsi else []
                    mm_waits = list(si.on_wait) if si else []
                    mm_upd = list(si.on_update) if si else []
                    inst.sync_info = mybir.SyncInfo(
                        on_wait=lw_waits + mm_waits, on_update=mm_upd)
                    inst.ldweights = True
                    inst.merge_dependencies_from(pending)
                    rename[pending.name] = inst.name
                    pending = None
                out.append(inst)
            assert pending is None
            blk.instructions = out
    if rename:
        for fn in nc.m.functions:
            for blk in fn.blocks:
                for inst in blk.instructions:
                    inst.remap_dependency_names(rename)


def build_nc() -> bacc.Bacc:
    nc = bacc.Bacc("TRN2", target_bir_lowering=False, debug=False)

    in_d = nc.dram_tensor("inp", [P, 4 * HL], BF16, kind="ExternalInput").ap()
    out_d = nc.dram_tensor("out", [P, 4], F32, kind="ExternalOutput").ap()
    w_d = nc.inline_tensor(_selector_weights(), name="selw").ap()
    mask_np = np.ones((32, NPP), dtype=NP_BF16)
    mask_np[31, NPP - 4:NPP] = 0.0
    mask_d = nc.inline_tensor(mask_np, name="mask").ap()

    with tile.TileContext(nc) as tc:
        with (
            tc.tile_pool(name="consts", bufs=1) as consts,
            tc.tile_pool(name="ins", bufs=1) as ins,
            tc.tile_pool(name="prods", bufs=2) as prods,
            tc.tile_pool(name="fin", bufs=1) as fin,
            tc.tile_pool(name="psum", bufs=2, space="PSUM") as psum,
            tc.tile_pool(name="wps", bufs=1, space="PSUM") as wps,
            tc.tile_pool(name="dram", bufs=1, space="DRAM") as dram,
        ):
            # ---- selector weights first (tiny; PE warmup needs them),
            # then the 8 group loads, all on the SP ring.
            w_sb = consts.tile([P, NS * 8], BF16)
            nc.sync.dma_start(w_sb[:, :], w_d)
            w3 = w_sb.rearrange("p (s m) -> p s m", s=NS)
            wsel = [w3[:, s] for s in range(NS)]

            tins = []
            for g in range(NG):
                t_in = ins.tile([P, 4 * FD], BF16, name=f"in{g}")
                src = bass.AP(
                    tensor=in_d.tensor,
                    offset=g * 4 * FD,
                    ap=[[4 * HL, P], [1, 4 * FD]],
                )
                nc.sync.dma_start(t_in[:, :], src)
                tins.append(t_in)

            # memsets on DVE (it is idle until the first products ~6us)
            scr = dram.tile([NQ, SCR_W], BF16)
            zeros = consts.tile([1, 16], BF16)
            nc.vector.memset(zeros[:, :], 0.0)
            scr_pad = bass.AP(
                tensor=scr.tensor,
                offset=scr.offset + L,
                ap=[[SCR_W, NQ], [1, HALO]],
            )
            nc.sync.dma_start(scr_pad, zeros[:, :])

            # stage-2 mask for the last chunk (l >= L-4 invalid).
            mask = consts.tile([32, NPP], BF16)
            nc.sync.dma_start(mask[:, :], mask_d)

            # ---- PE p-state warmup during the load window.
            wtile = consts.tile([P, CH], BF16)
            nc.vector.memset(wtile[:, :], 0.25)
            ps_w = wps.tile([8, CH], F32)
            for _ in range(14):
                nc.tensor.matmul(
                    ps_w[:, :], wsel[0], wtile[:, :], start=True, stop=True,
                )

            # pre-warm ACT's sqrt AND square tables off the critical path.
            warm = consts.tile([P, 2], F32)
            nc.vector.memset(warm[:, :], 1.0)
            nc.scalar.sqrt(warm[:, 0:1], warm[:, 0:1])
            nc.scalar.square(warm[:, 1:2], warm[:, 1:2])

            # ---- main loop.  Group block plane order is [pr, ti, pi, tr]
            # so every DVE mul's src0 (pr or pi) sits at an even-2KB SBUF
            # offset - the DVE 2x pipe mode needs that.
            def emit_group(g):
                t_in = tins[g]
                pl = [t_in[:, j * FD:(j + 1) * FD] for j in range(4)]
                pr, ti, pi, tr = pl

                def ptile(nm):
                    return prods.tile([P, FD], BF16, name=nm, tag=nm,
                                      padded_shape=[P, 2 * FD])
                movs = [ptile(f"s{s}") for s in range(NS)]
                # DVE: the four cross products (bf16 2x mode)
                nc.vector.tensor_mul(movs[0][:, :], pr, tr)
                nc.vector.tensor_mul(movs[1][:, :], pi, ti)
                nc.vector.tensor_mul(movs[2][:, :], pi, tr)
                nc.vector.tensor_mul(movs[3][:, :], pr, ti)
                # ACT: 2.5 squares; GPS: 1.5 squares
                nc.scalar.square(movs[4][:, :], pr)
                nc.scalar.square(movs[5][:, 0:CH], pi[:, 0:CH])
                nc.scalar.square(movs[6][:, :], tr)
                nc.gpsimd.tensor_mul(movs[5][:, CH:FD], pi[:, CH:FD],
                                     pi[:, CH:FD])
                nc.gpsimd.tensor_mul(movs[7][:, :], ti, ti)

                ps = psum.tile([8, FD], F32, name="ps", tag="ps")
                for si in range(NS):
                    for kk in range(0, FD, CH):
                        nc.tensor.matmul(
                            ps[:, kk:kk + CH], wsel[si],
                            movs[si][:, kk:kk + CH],
                            start=(si == 0), stop=(si == NS - 1),
                        )
                # drain psum -> bf16 relay (split ACT/GPS), then 2 scatter
                # DMAs: row 4h+q -> scr[q, 8192h + 1024g + n]
                dr = prods.tile([8, FD], BF16, name="dr", tag="dr")
                nc.vector.tensor_copy(dr[:, 0:CH], ps[:, 0:CH])
                nc.scalar.copy(dr[:, CH:FD], ps[:, CH:FD])
                for h in range(2):
                    dst = bass.AP(
                        tensor=scr.tensor,
                        offset=scr.offset + h * HL + g * FD,
                        ap=[[SCR_W, NQ], [1, FD]],
                    )
                    nc.sync.dma_start(dst, dr[4 * h:4 * h + 4, :])

            # stage-2 chunks (l0, Pn): two big ones fire after group 6,
            # two small ones (the only g7-dependent l-ranges) at the end.
            CHUNKS = [(0, 96), (HL, 96), (6144, 32), (14336, 32)]

            def emit_chunk(c):
                l0, pn = CHUNKS[c]
                stg = fin.tile([pn, NQ * (NPP + HALO)], BF16, name=f"stg{c}")
                sv = stg.rearrange("p (q n) -> p q n", q=NQ)
                src = bass.AP(
                    tensor=scr.tensor,
                    offset=scr.offset + l0,
                    ap=[[NPP, pn], [SCR_W, NQ], [1, NPP + HALO]],
                )
                nc.sync.dma_start(sv, src)

                win = fin.tile([pn, NQ * NPP], BF16, name=f"win{c}")
                wv = win.rearrange("p (q n) -> p q n", q=NQ)
                nc.vector.tensor_add(wv, sv[:, :, 0:NPP], sv[:, :, 1:NPP + 1])
                for j in range(2, K):
                    nc.vector.tensor_add(wv, wv, sv[:, :, j:NPP + j])

                def winq(q):
                    return win[:, q * NPP:(q + 1) * NPP]
                n2 = fin.tile([pn, NPP], BF16, name=f"n2_{c}")
                t2 = fin.tile([pn, NPP], BF16, name=f"t2_{c}")
                d2 = fin.tile([pn, NPP], BF16, name=f"d2_{c}")
                rd = fin.tile([pn, NPP], F32, name=f"rd_{c}")
                nc.vector.tensor_mul(n2[:, :], winq(0), winq(0))
                nc.vector.tensor_mul(t2[:, :], winq(1), winq(1))
                nc.vector.tensor_add(n2[:, :], n2[:, :], t2[:, :])
                nc.vector.tensor_mul(d2[:, :], winq(2), winq(3))
                nc.vector.reciprocal(rd[:, :], d2[:, :])
                nc.vector.tensor_mul(n2[:, :], n2[:, :], rd[:, :])
                if c == 3:
                    nc.vector.tensor_mul(n2[:, :], n2[:, :], mask[:, :])
                sq = fin.tile([pn, NPP], F32, name=f"sq{c}")
                acc = fin.tile([pn, 1], F32, name=f"acc{c}")
                nc.scalar.activation(
                    sq[:, :], n2[:, :],
                    mybir.ActivationFunctionType.Sqrt,
                    accum_out=acc[:, :],
                )
                nc.scalar.dma_start(out_d[0:pn, c:c + 1], acc[:, :])

            for g in range(NG):
                emit_group(g)
                if g == 6:
                    emit_chunk(0)
                    emit_chunk(1)
            emit_chunk(2)
            emit_chunk(3)

    # bf16 matmuls keep the split Ldweights+Matmult form: the standalone
    # Ldweights pipelines with the previous matmul; fusing (needed only
    # for fp8 DoubleRow) serializes the weight load into every matmul.
    nc.compile()
    return nc


_NC = None


def _get_nc() -> bacc.Bacc:
    global _NC
    if _NC is None:
        _NC = build_nc()
    return _NC


def kernel(pred_real, pred_imag, targ_real, targ_imag, filter_size=5):
    global LAST_RESULT
    assert int(filter_size) == K
    nc = _get_nc()

    in_maps = []
    for b in range(B):
        # plane-separate: partition p = c + 64h, plane cols = l % 8192;
        # group-blocked [PR | TI | PI | TR] per 1024-col stripe (order
        # keeps every DVE mul's src0 at an even-2KB SBUF offset).
        arr = np.empty((P, 4 * HL), dtype=NP_BF16)
        a4 = arr.reshape(P, NG, 4, FD)
        for j, x in zip((0, 2, 3, 1), (pred_real[b], pred_imag[b],
                                       targ_real[b], targ_imag[b])):
            xp = np.asarray(x, dtype=np.float32).reshape(C, 2, NG, FD)
            # -> [h*64 + c, g, n]
            a4[:, :, j, :] = xp.transpose(1, 0, 2, 3).reshape(P, NG, FD) \
                .astype(NP_BF16)
        in_maps.append({"inp": arr})

    kwargs = {}
    if PROFILE:
        kwargs = dict(trace=True)
        if TRACE_DIR is not None:
            import os
            os.makedirs(TRACE_DIR, exist_ok=True)
            kwargs["tmpdir"] = TRACE_DIR
    res = run_bass_kernel_spmd(nc, in_maps, core_ids=list(range(B)), **kwargs)
    LAST_RESULT = res

    total = 0.0
    for r in res.results:
        o = np.asarray(r["out"], dtype=np.float64)
        for c, pn in enumerate((96, 96, 32, 32)):
            total += o[0:pn, c].sum()
    coh = total / (B * NVALID)
    return np.float32(1.0 - coh)
